# revision 72
# baseline (speedup 1.0000x reference)
"""Trainium2 Bass kernel for nn_Attention_18863496364032 (self-contained).

fused attention block: qkv proj -> 16-head scaled-dot-product attention ->
out proj + bias -> LayerNorm, for x [4, 2048, 1024] f32.

Sharding: core c handles batch b = c//2 and head-group g = c%2 (8 of the 16
heads) over ALL 2048 tokens of its batch, so K/V/Q projections are computed
only for the core's own heads (no duplicated projection work). After each
head-pair's attention is normalized, the two cores of a pair exchange the
outputs for each other's token half via a small pairwise AllGather (256 KB,
runs on TOPSP/SDMA, overlapped with compute). Each core then runs the
out-projection + bias + LayerNorm for its own 1024-token half with the full
16-head contraction.

SPMD symmetry tricks (one program serves all 8 cores):
 - odd cores see their batch's tokens rotated by 1024 on the host, so "my
   token half" is always local columns 0:1024;
 - the AllGather output carries both pair members' contributions; host-fed
   0/1 scalars select the peer's rows on DVE;
 - w_out rows are host-reordered per core ([my heads | peer heads]).
"""

import numpy as np
import ml_dtypes

import concourse.bass as bass
import concourse.mybir as mybir
import concourse.tile as tile
from concourse.bass_utils import run_bass_kernel_spmd
from concourse.vector_clock import ScopedClock

BF16 = mybir.dt.bfloat16
F32 = mybir.dt.float32
AF = mybir.ActivationFunctionType
ALU = mybir.AluOpType

# ---------------------------------------------------------------------------
# Workarounds for the container toolchain (walrus rejects >1 sync-wait per
# instruction; the Tile end-of-kernel drain carries several).
# ---------------------------------------------------------------------------


def _drain_and_barrier_split(self, tick_clock, wait_clock):
    nc = self.nc
    probe = nc.sync.nop()
    wait_clock.add_sem_waits(probe.ins, ScopedClock({None: tick_clock.global_clock}))
    si = probe.ins.sync_info
    waits = list(si.on_wait) if si is not None and si.on_wait else []
    if len(waits) > 1:
        probe.ins.sync_info = mybir.SyncInfo(on_wait=waits[:1], on_update=[])
        for w in waits[1:]:
            extra = nc.sync.nop()
            extra.ins.sync_info = mybir.SyncInfo(on_wait=[w], on_update=[])
    nc.sync.drain()

    nc.all_engine_barrier()
    assert self.sems is not None
    popped = nc._tile_sem_poison_stack.pop()
    assert popped is self._sem_poison
    nc.clear_and_free_semaphores(list(self.sems.allocated().values()))
    nc.all_engine_barrier()


tile.TileContext._drain_and_barrier = _drain_and_barrier_split

_nsplit = [0]


def split_excess_waits(nc, max_waits=1):
    """Hoist excess sync waits onto same-engine nops placed before."""
    n = 0
    for f in nc.m.functions:
        for blk in f.blocks:
            out = []
            changed = False
            for inst in blk.instructions:
                si = inst.sync_info
                waits = list(si.on_wait) if si is not None and si.on_wait else []
                if len(waits) > max_waits:
                    changed = True
                    extra, keep = waits[:-max_waits], waits[-max_waits:]
                    for i in range(0, len(extra), max_waits):
                        _nsplit[0] += 1
                        n += 1
                        nop = mybir.InstNoOp(
                            name=f"I-waitsplit-{_nsplit[0]}", ins=[], outs=[])
                        nop.engine = inst.engine
                        nop.sync_info = mybir.SyncInfo(
                            on_wait=extra[i:i + max_waits], on_update=[])
                        out.append(nop)
                    inst.sync_info = mybir.SyncInfo(
                        on_wait=keep,
                        on_update=list(si.on_update) if si.on_update else [])
                out.append(inst)
            if changed:
                blk.instructions = out
    return n


# ---------------------------------------------------------------------------
# Kernel builder
# ---------------------------------------------------------------------------


def _bcast_ap(ap, p=128):
    # replicate a [N] dram tensor across p partitions during DMA
    return bass.AP(tensor=ap.tensor, offset=ap.offset, ap=[[0, p]] + list(ap.ap))


def build_nc(apply_gamma=True, apply_beta=True, DIM=1024, N=2048, EPS=1e-5,
             debug=False):
    D = 64
    HL = 8                 # local heads per core
    HP = HL // 2           # local head pairs (c-tiles of its q/k)
    DT = DIM // 128        # contraction tiles over model dim
    TT = N // 128          # key tiles
    TQ = N // 2            # my-token-half size
    NQT = TQ // 128        # output token tiles
    LAG = 5
    scale = float(D) ** -0.5

    def fblocks(total, blk=512):
        return [(i, min(blk, total - i)) for i in range(0, total, blk)]

    IH = fblocks(N, 512)   # query blocks (all 2048 local-order tokens)
    OB = fblocks(DIM, 512)

    nc = bass.Bass()
    x_d = nc.declare_dram_parameter("xT", [DIM, N], BF16, isOutput=False)
    wq_d = nc.declare_dram_parameter("w_q", [DIM, HL * D], BF16, isOutput=False)
    wk_d = nc.declare_dram_parameter("w_k", [DIM, HL * D], BF16, isOutput=False)
    wv_d = nc.declare_dram_parameter("w_v", [DIM, HL * D], BF16, isOutput=False)
    wo_d = nc.declare_dram_parameter("wo_re", [2 * HL * D, DIM], BF16,
                                     isOutput=False)
    bo_d = nc.declare_dram_parameter("b_out", [DIM], BF16, isOutput=False)
    id_d = nc.declare_dram_parameter("ident", [128, 128], BF16, isOutput=False)
    s0_d = nc.declare_dram_parameter("sel0", [128, 1], F32, isOutput=False)
    s1_d = nc.declare_dram_parameter("sel1", [128, 1], F32, isOutput=False)
    if apply_gamma:
        g_d = nc.declare_dram_parameter("ln_gamma", [DIM], F32, isOutput=False)
    if apply_beta:
        be_d = nc.declare_dram_parameter("ln_beta", [DIM], F32, isOutput=False)
    y_d = nc.declare_dram_parameter("y", [TQ, DIM], F32, isOutput=True)
    if debug:
        dbg_ao = [nc.declare_dram_parameter(f"dbg_ao{hp}", [128, N], F32,
                                            isOutput=True) for hp in range(4)]
        dbg_sel = [nc.declare_dram_parameter(f"dbg_sel{hp}", [128, TQ], F32,
                                             isOutput=True) for hp in range(4)]
        dbg_ya = [nc.declare_dram_parameter(f"dbg_ya{t}", [128, DIM], F32,
                                            isOutput=True) for t in range(8)]
        dbg_kq = [nc.declare_dram_parameter(f"dbg_{n}", [128, N], F32,
                                            isOutput=True) for n in ("k3", "q3")]
        dbg_v = nc.declare_dram_parameter("dbg_v0", [128, 8 * 65], F32,
                                          isOutput=True)

    rb_dram = {}
    for hp in range(HP):
        for i4 in range(4):
            for sub in range(2):
                rb_dram[(hp, i4, sub)] = nc.dram_tensor(
                    f"rbd{hp}_{i4}_{sub}", [1, 512], BF16, kind="Internal")
    cc_in = [nc.dram_tensor(f"cc_in{hp}", [128, TQ], BF16, kind="Internal")
             for hp in range(HP)]
    cc_out = [nc.dram_tensor(f"cc_out{hp}", [256, TQ], BF16, kind="Internal")
              for hp in range(HP)]
    RG = [[0, 1], [2, 3], [4, 5], [6, 7]]

    with tile.TileContext(nc) as tc:
        with (
            tc.tile_pool(name="consts", bufs=1) as consts,
            tc.tile_pool(name="persist", bufs=1) as persist,
            tc.tile_pool(name="kqrot", bufs=2) as kqrot,
            tc.tile_pool(name="recvrot", bufs=3) as recvrot,
            tc.tile_pool(name="ps_s", bufs=3, space="PSUM") as ps_s,
            tc.tile_pool(name="ppool", bufs=9) as ppool,
            tc.tile_pool(name="work", bufs=3) as work,
            tc.tile_pool(name="fastn", bufs=2) as fastn,
            tc.tile_pool(name="yout", bufs=2) as yout,
        ):
            ps_ao = tc.alloc_tile_pool(name="ps_ao", bufs=2, space="PSUM")
            ones_row = consts.tile([1, 128], BF16, tag="ones_row", name="ones_row")
            nc.vector.memset(ones_row, 1.0)
            eps_sb = consts.tile([128, 1], F32, tag="eps", name="eps")
            nc.vector.memset(eps_sb, EPS)

            # ---- DMA wave 1: x + projection weights (ordered by first use)
            xw = tc.alloc_tile_pool(name="xw", bufs=1)
            xs = [xw.tile([128, N], BF16, tag=f"x{d}", name=f"x{d}")
                  for d in range(DT)]
            wq = [xw.tile([128, HL * D], BF16, tag=f"wq{d}", name=f"wq{d}")
                  for d in range(DT)]
            wk = [xw.tile([128, HL * D], BF16, tag=f"wk{d}", name=f"wk{d}")
                  for d in range(DT)]
            wv = [xw.tile([128, HL * D], BF16, tag=f"wv{d}", name=f"wv{d}")
                  for d in range(DT)]
            # small first-use weight columns FIRST (they gate the same
            # matmuls as x but cost only ~0.5MB), then x spread across
            # three queues (sync/scalar/gpsimd) for ~3us aggregate latency
            for d in range(DT):
                r = slice(d * 128, (d + 1) * 128)
                nc.sync.dma_start(out=wk[d][:, 0:128], in_=wk_d[r, 0:128])
                nc.gpsimd.dma_start(out=wq[d][:, 0:128], in_=wq_d[r, 0:128])
            qs = [nc.sync, nc.scalar, nc.gpsimd]
            for d in range(DT):
                r = slice(d * 128, (d + 1) * 128)
                qs[d % 3].dma_start(out=xs[d], in_=x_d[r, :])
            for d in range(DT):
                r = slice(d * 128, (d + 1) * 128)
                nc.scalar.dma_start(out=wv[d], in_=wv_d[r, :])
            for d in range(DT):
                r = slice(d * 128, (d + 1) * 128)
                nc.sync.dma_start(out=wk[d][:, 128:], in_=wk_d[r, 128:])
                nc.sync.dma_start(out=wq[d][:, 128:], in_=wq_d[r, 128:])

            # ---- DMA wave 2: out-proj / LN / misc constants
            bo_b = consts.tile([128, DIM], BF16, tag="bo", name="bo")
            nc.sync.dma_start(out=bo_b, in_=_bcast_ap(bo_d[:]))
            ident = consts.tile([128, 128], BF16, tag="ident", name="ident")
            nc.sync.dma_start(out=ident, in_=id_d[:])
            s0_sb = consts.tile([128, 1], F32, tag="s0", name="s0")
            nc.sync.dma_start(out=s0_sb, in_=s0_d[:])
            s1_sb = consts.tile([128, 1], F32, tag="s1", name="s1")
            nc.sync.dma_start(out=s1_sb, in_=s1_d[:])
            if apply_gamma:
                gamma_b = consts.tile([128, DIM], F32, tag="gamma", name="gamma")
                nc.sync.dma_start(out=gamma_b, in_=_bcast_ap(g_d[:]))
            if apply_beta:
                beta_b = consts.tile([128, DIM], F32, tag="beta", name="beta")
                nc.sync.dma_start(out=beta_b, in_=_bcast_ap(be_d[:]))
            wo = [persist.tile([128, DIM], BF16, tag=f"wo{j}", name=f"wo{j}")
                  for j in range(2 * HP)]
            for j in range(2 * HP):
                nc.sync.dma_start(out=wo[j], in_=wo_d[j * 128:(j + 1) * 128, :])

            # ---- persistent state
            # v layout: per local head 65 columns = [v_h (64) | ones]
            v_t = [persist.tile([128, HL * 65], BF16, tag=f"v{t}", name=f"v{t}")
                   for t in range(TT)]
            aoT = [persist.tile([128, N], BF16, tag=f"aoT{c}", name=f"aoT{c}")
                   for c in range(HP)]
            sel = [persist.tile([128, TQ], BF16, tag=f"sel{c}", name=f"sel{c}")
                   for c in range(HP)]
            y_acc = [persist.tile([128, DIM], BF16, tag=f"ya{t}", name=f"ya{t}")
                     for t in range(NQT)]

            # ---------- production tasks ----------
            def v_task(t):
                def run():
                    ps = ps_s.tile([128, 1024], F32, tag="sp", name="prv")
                    for d in range(DT):
                        nc.tensor.matmul(
                            ps[:, 0:HL * D],
                            lhsT=xs[d][:, t * 128:(t + 1) * 128],
                            rhs=wv[d],
                            start=(d == 0), stop=(d == DT - 1),
                        )
                    dst = v_t[t].rearrange("p (h e) -> p h e", e=65)
                    nc.vector.tensor_copy(
                        dst[:, :, 0:64],
                        ps[:, 0:HL * D].rearrange("p (h e) -> p h e", e=64),
                    )
                    nc.vector.memset(dst[:, :, 64:65], 1.0)
                return run

            def kq_task(kt, hp, tb, w, wsrc):
                def run():
                    ps = ps_s.tile([128, 1024], F32, tag="sp", name="prk")
                    for d in range(DT):
                        nc.tensor.matmul(
                            ps[:, :w],
                            lhsT=wsrc[d][:, hp * 128:(hp + 1) * 128],
                            rhs=xs[d][:, tb:tb + w],
                            start=(d == 0), stop=(d == DT - 1),
                        )
                    nc.vector.tensor_copy(kt[:, tb:tb + w], ps[:, :w])
                return run

            def kq_tasks(hp):
                kt = kqrot.tile([128, N], BF16, tag="kT", name=f"kT{hp}")
                qt = kqrot.tile([128, N], BF16, tag="qT", name=f"qT{hp}")
                ktasks = [kq_task(kt, hp, tb, w, wk) for tb, w in fblocks(N)]
                qtasks = [kq_task(qt, hp, tb, w, wq) for tb, w in fblocks(N)]
                return kt, qt, ktasks, qtasks

            # PE warmup: dependency-free matmuls un-throttle the HAM clock
            # while the first DMA wave lands.
            wps = ps_s.tile([128, 1024], F32, tag="sp", name="warm")
            for _ in range(24):
                nc.tensor.matmul(
                    wps[0:128, 0:128], lhsT=ones_row, rhs=ones_row[:, 0:128],
                    start=True, stop=True,
                )
            nc.vector.tensor_copy(
                work.tile([128, 128], F32, tag="wsink", name="wsink"),
                wps[:, 0:128])

            # Scores start as soon as kT0 block 0 + qT0 block 2 land; all
            # other production (remaining kT/qT blocks of every round, all
            # v tiles) runs just-in-time inside the chunk stream, keyed by
            # absolute stream slot. Per round hp the slot map carries:
            #   kT[hp] blocks 1..3 at slots base+1/5/9 (block b first read
            #   at chunk 4b of every ih-block), qT[hp] blocks for upcoming
            #   ih positions at slots base+7/11/13, and each round's lead
            #   kT block 0 + lead qT block created at kq_lead slots.
            kt_cur, qt_cur, kt0, qt0 = kq_tasks(0)
            kt0[0]()
            qt0[2]()  # blocks run in order (2,3,0,1)
            slot_tasks = {}

            def add_task(s, task):
                slot_tasks.setdefault(s, []).append(task)

            def schedule_kq_round(base, ktasks, qtasks):
                # called for a round's own kT/qT blocks (except kT[0] and
                # qT[2], produced earlier at the kq_lead slot)
                for k, bidx in enumerate((1, 2, 3)):
                    add_task(base + 4 * k + 1, ktasks[bidx])
                for k, bidx in enumerate((3, 0, 1)):
                    add_task(base + 2 * k + 7, qtasks[bidx])

            schedule_kq_round(0, kt0, qt0)
            # ten v tiles up front: dense early PE work keeps the HAM clock
            # governor at full speed (a sparse start latches a ~17% lower
            # clock for the entire run); the rest go just-in-time (issued
            # at or before their first reader's slot -- Tile dependencies
            # are program-order-based).
            for t in range(TT):
                add_task(t // 2 if t < 10 else t - 5, v_task(t))

            def norm_piece(hp, sub, aot, ioff, iw):
                # normalize: aoT[hp] rows 0:64 = head 2hp, rows 64:128 head
                # 2hp+1; softmax denominator sits in psum row 64 (ones col
                # of v_aug).
                def run():
                    # copy the whole ao tile (64 dims + denominator row) out
                    # of PSUM first -- this frees the ao rotation slot for
                    # the next block's AVs in one op; the reciprocal chain
                    # then runs off the critical path. DVE reciprocal costs
                    # ~12 cycles/free-element regardless of partitions, so
                    # reshape the [1,512] denominator row across 128
                    # partitions (tiny SBUF<->SBUF DMAs) to make it ~0.2us.
                    aou = work.tile([65, 512], F32, tag="aou", name="aou")
                    nc.vector.tensor_copy(aou[:, :iw], aot[0:65, 0:iw])
                    dcoll = work.tile([128, 4], F32, tag="dcoll", name="dcoll")
                    nc.sync.dma_start(out=dcoll, in_=aou[64:65, :iw])
                    rcoll = work.tile([128, 4], F32, tag="rcoll", name="rcoll")
                    nc.vector.reciprocal(rcoll, dcoll)
                    rb128 = work.tile([128, 4], BF16, tag="rb128", name="rb128")
                    nc.vector.tensor_copy(rb128, rcoll)
                    # bounce 1/den through DRAM so a stride-0-source DMA
                    # can broadcast it across 64 partitions -- no PSUM, no
                    # PE matmul, no compute engine involved
                    rbd = rb_dram[(hp, ioff // 512, sub)]
                    nc.sync.dma_start(out=rbd[:, 0:iw], in_=rb128)
                    rbs = work.tile([64, 512], BF16, tag="rbs", name="rbs")
                    nc.sync.dma_start(out=rbs[:, :iw],
                                      in_=_bcast_ap(rbd[0, 0:iw], p=64))
                    # final normalize multiply on Pool (SBUF-only op): keeps
                    # DVE free for the PSUM-copy traffic it alone can do
                    nc.gpsimd.tensor_mul(
                        aoT[hp][sub * 64:sub * 64 + 64, ioff:ioff + iw],
                        aou[0:64, 0:iw], rbs[:, :iw])
                return run

            def norm_piece_fast(hp, sub, aot, ioff, iw, shared):
                # latency-optimized norm: 1/den = exp(-ln(den)) on ACT
                # straight off the [1,iw] row, PE ones-broadcast to 64
                # partitions via PSUM, DVE mult. No DMA, no DRAM bounce.
                # Split in two parts: A (aou copy + ACT recip) at the block
                # end, B (PE broadcast + mult) a couple slots later so the
                # PE never stalls waiting for ACT to produce rrow.
                state = {}

                def part_a():
                    aou = work.tile([65, 512], F32, tag="aou", name="aouF")
                    nc.vector.tensor_copy(aou[:, :iw], aot[0:65, 0:iw])
                    lnd = fastn.tile([1, 512], F32, tag="lnd", name="lnd")
                    nc.scalar.activation(lnd[:, :iw], aou[64:65, :iw], AF.Ln)
                    rrow = fastn.tile([1, 512], BF16, tag="rrow", name="rrow")
                    nc.scalar.activation(rrow[:, :iw], lnd[:, :iw], AF.Exp,
                                         scale=-1.0)
                    state["t"] = (aou, rrow)

                def part_b():
                    aou, rrow = state["t"]
                    # both subs of a block share ONE psum ring tile (halves
                    # the extra ring traffic that stalls the broadcast)
                    if "rb" not in shared:
                        shared["rb"] = ps_s.tile([128, 1024], F32,
                                                 tag="sp", name="rbbc")
                    rb = shared["rb"]
                    col = sub * 512
                    nc.tensor.matmul(rb[0:64, col:col + iw],
                                     lhsT=ones_row[:, 0:64],
                                     rhs=rrow[:, :iw], start=True, stop=True)
                    nc.vector.tensor_mul(
                        aoT[hp][sub * 64:sub * 64 + 64, ioff:ioff + iw],
                        aou[0:64, 0:iw], rb[0:64, col:col + iw])
                return part_a, part_b

            def send_tasks(ci, co, src_lo, w):
                # bounce out my-heads data for the peer's token range, then
                # the pairwise AllGather (runs on TOPSP/SDMA). The bounce is
                # split into 4 DMAs so they land on different hw queues and
                # transfer in parallel (~4x faster than one 256KB DMA).
                def t_out():
                    for k in range(4):
                        c0 = k * (w // 4)
                        nc.sync.dma_start(out=ci[:, c0:c0 + w // 4],
                                          in_=src_lo[:, c0:c0 + w // 4])

                def t_cc():
                    nc.gpsimd.collective_compute(
                        "AllGather", ALU.bypass,
                        ins=[ci[:]], outs=[co[:]],
                        replica_groups=RG,
                    )
                return [t_out, t_cc]

            def recv_task(co, hp, off, w):
                # recv DMAs ride the Pool hwdge queue: Pool runs far ahead
                # of wall-clock (only the light norm-multiplies), so the
                # recv issues promptly; its Collectives-sem wait only
                # delays later Pool work (the next norms), which has slack.
                # ACT's queue would drain ~100us late; Sync/DVE queues
                # would stall critical traffic behind the AG wait.
                holder = {}

                def t_in():
                    # ONE recv DMA for both AG blocks: the single transfer
                    # carries the Collectives-sem wait, so neither half can
                    # race ahead of the AllGather (two separate DMAs land on
                    # different hw queues and only the first gets the wait)
                    r01 = recvrot.tile([128, 2 * TQ], BF16, tag="r0",
                                       name=f"r01_{hp}")
                    nc.gpsimd.dma_start(
                        out=r01,
                        in_=co[0:256, 0:w].rearrange("(b p) q -> p b q", b=2))
                    holder["r"] = (r01[:, 0:TQ], r01[:, TQ:2 * TQ])

                def t_sel():
                    # select math stays on DVE (walrus rejects AP-scalar
                    # TensorScalar on Pool); slotted away from kq clusters
                    r0, r1 = holder["r"]
                    tmp = recvrot.tile([128, TQ], BF16, tag="seltmp",
                                       name=f"st{hp}")
                    nc.vector.tensor_scalar_mul(tmp[:, 0:w], r0[:, 0:w],
                                                s0_sb)
                    nc.vector.scalar_tensor_tensor(
                        out=sel[hp][:, off:off + w], in0=r1[:, 0:w],
                        scalar=s1_sb, in1=tmp[:, 0:w],
                        op0=ALU.mult, op1=ALU.add)
                return t_in, t_sel

            def oproj_pass1(t):
                # y_acc[t] = bias + sum of c-tile contributions from
                # head-pairs 0..2 (local + received)
                def run():
                    ps = ps_s.tile([128, 1024], F32, tag="sp", name="op1")
                    # only hp0+hp1 here: everything is ready by slot ~172,
                    # so pops spread thin with zero stalls; hp2/hp3 (whose
                    # aoT-own norms and recvs land late) join the tail pass
                    srcs = [(aoT[0], 0), (sel[0], HP), (aoT[1], 1),
                            (sel[1], HP + 1)]
                    mms = [(src, j, off, w)
                           for src, j in srcs for off, w in OB]
                    for i, (src, j, off, w) in enumerate(mms):
                        nc.tensor.matmul(
                            ps[:, off:off + w],
                            lhsT=src[:, t * 128:(t + 1) * 128],
                            rhs=wo[j][:, off:off + w],
                            start=(i < 2), stop=(i >= len(mms) - 2),
                        )
                    nc.vector.scalar_tensor_tensor(
                        out=y_acc[t], in0=ps, scalar=1.0, in1=bo_b,
                        op0=ALU.mult, op1=ALU.add)
                return run

            def tail_tile(t):
                # last head-pair (local + received) + hp2's received half
                # + accumulated partial
                ps = ps_s.tile([128, 1024], F32, tag="sp", name="op2")
                hp = HP - 1
                mms = [(aoT[hp][:, t * 128:(t + 1) * 128], hp, off, w)
                       for off, w in OB]
                mms += [(sel[hp][:, t * 128:(t + 1) * 128], 2 * HP - 1, off, w)
                        for off, w in OB]
                mms += [(aoT[2][:, t * 128:(t + 1) * 128], 2, off, w)
                        for off, w in OB]
                mms += [(sel[2][:, t * 128:(t + 1) * 128], HP + 2, off, w)
                        for off, w in OB]
                for i, (lhs, j, off, w) in enumerate(mms):
                    nc.tensor.matmul(
                        ps[:, off:off + w], lhsT=lhs,
                        rhs=wo[j][:, off:off + w],
                        start=(i < 2), stop=False,
                    )
                for off, w in OB:
                    nc.tensor.matmul(
                        ps[:, off:off + w], lhsT=ident,
                        rhs=y_acc[t][:, off:off + w],
                        start=False, stop=True,
                    )
                ng = (DIM + 511) // 512
                st = work.tile([128, ng, 6], F32, tag="bnst", name="bnst")
                for gi in range(ng):
                    gw = min(512, DIM - gi * 512)
                    nc.vector.bn_stats(st[:, gi, :],
                                       ps[:, gi * 512:gi * 512 + gw])
                mv = work.tile([128, 2], F32, tag="mv", name="mv")
                nc.vector.bn_aggr(mv, st)
                # rstd = exp(-0.5*ln(var+eps)); Ln+Exp share one ACT table
                lnv = work.tile([128, 1], F32, tag="lnv", name="lnv")
                nc.scalar.activation(lnv, mv[:, 1:2], AF.Ln, bias=eps_sb)
                rstd = work.tile([128, 1], F32, tag="rstd", name="rstd")
                nc.scalar.activation(rstd, lnv, AF.Exp, scale=-0.5)
                nmr = work.tile([128, 1], F32, tag="nmr", name="nmr")
                nc.vector.tensor_scalar(
                    nmr, mv[:, 0:1], scalar1=rstd, scalar2=-1.0,
                    op0=ALU.mult, op1=ALU.mult,
                )
                yA = yout.tile([128, DIM], F32, tag="yA", name="yA")
                nc.scalar.activation(yA, ps, AF.Identity, bias=nmr, scale=rstd)
                cur = yA
                if apply_gamma:
                    yn = yout.tile([128, DIM], F32, tag="yn", name="yn")
                    nc.vector.tensor_mul(yn, cur, gamma_b)
                    cur = yn
                if apply_beta:
                    yn2 = yout.tile([128, DIM], F32, tag="yA", name="yn2")
                    nc.gpsimd.tensor_add(yn2, cur, beta_b)
                    cur = yn2
                nc.sync.dma_start(out=y_d[t * 128:(t + 1) * 128, :], in_=cur)

            # ---------- main rounds: one continuous chunk stream ----------
            # scores/exp run LAG chunks ahead of AV with no drain at block
            # or round boundaries; ao-pair PSUM rotation (bufs=2) hands off
            # between consecutive blocks. Segment order interleaves the
            # last two head-pairs' peer-half blocks before their own-half
            # blocks, so ALL four exchanges launch and land mid-stream:
            #   hp0(2,3,0,1) hp1(2,3,0,1) hp2(2,3) hp3(2,3) hp2(0,1) hp3(0,1)
            # The tail's op2 (hp3) then finds sel3/aoT3 ready with no wait.
            deferred = []
            pending = []
            SEGS = [(0, (2, 3, 0, 1)), (1, (2, 3, 0, 1)),
                    (2, (2, 3)), (3, (2, 3)), (2, (0, 1)), (3, (0, 1))]
            chunks = []           # (hp, jt, ioff, iw)
            for hp_, blocks_ in SEGS:
                for i in blocks_:
                    ioff_, iw_ = IH[i]
                    for jt_ in range(TT):
                        chunks.append((hp_, jt_, ioff_, iw_))
            kqs = {0: (kt_cur, qt_cur)}
            # kq production for rounds 1..3 is pushed into task-free windows
            # (NOT at round starts, where it used to pile up on DVE/psum):
            # hp2 in late S1, hp3 in early S2. (slot, 'k'/'q', block) lists.
            kq_sched = {
                1: [(50, 'k', 0), (52, 'q', 2), (65, 'k', 1), (69, 'k', 2),
                    (73, 'k', 3), (71, 'q', 3), (75, 'q', 0), (77, 'q', 1)],
                2: [(78, 'k', 0), (82, 'q', 2), (86, 'k', 1), (94, 'k', 2),
                    (102, 'k', 3), (90, 'q', 3), (98, 'q', 0),
                    (106, 'q', 1)],
                3: [(129, 'k', 0), (133, 'q', 2), (137, 'k', 1),
                    (145, 'k', 2), (153, 'k', 3), (141, 'q', 3),
                    (149, 'q', 0), (157, 'q', 1)],
            }
            lead_at = {min(s for s, _, _ in v): k for k, v in kq_sched.items()}
            # op1 needs only hp0/hp1 data: sel1 select is emitted at slot
            # 184, so pops can start right after and spread thin
            OPROJ_AT = 194        # slot where op1 tasks become pending
            OP1_MIN = {t: 194 for t in range(8)}
            ex_slots = {}         # stream index -> task list

            def add_ex(s, task):
                ex_slots.setdefault(s, []).append(task)

            # send after both peer-half blocks' norm chains are done;
            # receive + select a fixed distance later (recv DMAs ride the
            # ACT hwdge queue; select runs on Pool)
            # sel emission slots track when each AG's data ACTUALLY lands
            # (measured): emitting earlier stalls the whole DVE queue behind
            # the recv wait; emitting later is free.
            ex_sched = {0: (40, 66, 136), 1: (104, 130, 192),
                        2: (168, 194, 256), 3: (200, 226, 264)}
            for hp in range(HP):
                snd, rcv, slt = ex_sched[hp]
                for k, task in enumerate(send_tasks(
                        cc_in[hp], cc_out[hp], aoT[hp][:, TQ:N], TQ)):
                    add_ex(snd + k, task)
                t_in, t_sel = recv_task(cc_out[hp], hp, 0, TQ)
                add_ex(rcv, t_in)
                add_ex(slt, t_sel)

            NCH = len(chunks)
            pq = []
            cur_ao = None
            av_pending = []
            for s in range(NCH + LAG + 4):
                # norm pieces first: they free the ao pair that this slot's
                # AV (new block, start=True) is about to reuse.
                while deferred:
                    deferred.pop(0)()
                if s < NCH:
                    hp, jt, ioff, iw = chunks[s]
                    if s in lead_at:
                        # a later round's kT/qT, scheduled per kq_sched
                        hp_n = lead_at[s]
                        kt_nxt, qt_nxt, ktn, qtn = kq_tasks(hp_n)
                        kqs[hp_n] = (kt_nxt, qt_nxt)
                        for slot, kind, blk in kq_sched[hp_n]:
                            add_task(slot, ktn[blk] if kind == 'k'
                                     else qtn[blk])
                    if s == OPROJ_AT:
                        # progressive out-projection over the final segments
                        pending.extend((OP1_MIN[t], oproj_pass1(t))
                                       for t in range(NQT))
                    kt_c, qt_c = kqs[hp]
                    sp = ps_s.tile([128, 1024], F32, tag="sp", name="sp")
                    nc.tensor.matmul(
                        sp[:, 0:iw],
                        lhsT=kt_c[0:64, jt * 128:(jt + 1) * 128],
                        rhs=qt_c[0:64, ioff:ioff + iw],
                        start=True, stop=True, tile_position=(0, 0),
                    )
                    nc.tensor.matmul(
                        sp[:, iw:2 * iw],
                        lhsT=kt_c[64:128, jt * 128:(jt + 1) * 128],
                        rhs=qt_c[64:128, ioff:ioff + iw],
                        start=True, stop=True, tile_position=(64, 0),
                    )
                    p = ppool.tile([128, 1024], BF16, tag="p", name="p")
                    nc.scalar.activation(p[:, 0:2 * iw], sp[:, 0:2 * iw],
                                         AF.Exp, scale=scale)
                    pq.append(p)
                    ran = False
                    for task in slot_tasks.pop(s, ()):
                        task()
                        ran = True
                    for task in ex_slots.get(s, ()):
                        task()
                        ran = True
                    if (not ran and jt % 3 == 1 and pending
                            and s >= pending[0][0]):
                        pending.pop(0)[1]()
                else:
                    # drain slots still run scheduled tasks (the delayed
                    # norm part_b's of the last blocks and the last sel
                    # land here)
                    for task in slot_tasks.pop(s, ()):
                        task()
                    for task in ex_slots.get(s, ()):
                        task()
                if LAG <= s < NCH + LAG:
                    # Batch AV emission in chunk pairs: the PE pays ~120ns
                    # per tile-config transition (row-split scores <-> full
                    # AVs), so [sp sp ao ao ao ao] halves the transitions.
                    # Block-end chunks (jt==15) flush solo so the norm
                    # part_a still precedes the next block's jt0 AV (psum
                    # pair handoff + send-slot safety).
                    av_pending.append(s - LAG)
                if av_pending and (chunks[av_pending[-1]][1] == TT - 1
                                   or len(av_pending) == 4
                                   or s >= NCH + LAG - 1):
                    for c in av_pending:
                        hp, jt, ioff, iw = chunks[c]
                        h0c, h1c = 2 * hp * 65, (2 * hp + 1) * 65
                        if jt == 0:
                            ao0 = ps_ao.tile([65, 512], F32, tag="ao",
                                             name="ao")
                            ao1 = ps_ao.tile([65, 512], F32, tag="ao",
                                             name="ao")
                            cur_ao = (ao0, ao1)
                        ao0, ao1 = cur_ao
                        p = pq.pop(0)
                        first, last = (jt == 0), (jt == TT - 1)
                        nc.tensor.matmul(
                            ao0[0:65, 0:iw],
                            lhsT=v_t[jt][:, h0c:h0c + 65],
                            rhs=p[:, 0:iw],
                            start=first, stop=last,
                        )
                        nc.tensor.matmul(
                            ao1[0:65, 0:iw],
                            lhsT=v_t[jt][:, h1c:h1c + 65],
                            rhs=p[:, iw:2 * iw],
                            start=first, stop=last,
                        )
                        if last:
                            # DMA-free fast norm: ACT has headroom, and it
                            # keeps the sync queue + DVE reciprocal chains
                            # out of the exchange critical path
                            shared = {}
                            a0, b0 = norm_piece_fast(hp, 0, ao0, ioff, iw,
                                                     shared)
                            a1, b1 = norm_piece_fast(hp, 1, ao1, ioff, iw,
                                                     shared)
                            deferred.append(a0)
                            deferred.append(a1)
                            add_task(s + 2, b0)
                            add_task(s + 3, b1)
                    av_pending.clear()

            # ---------- tail: last norms, out-proj + LN ----
            while deferred:
                deferred.pop(0)()
            while pending:
                pending.pop(0)[1]()
            xw.release()
            for t in range(NQT):
                tail_tile(t)
            ps_ao.release()
            if debug:
                dpool = tc.alloc_tile_pool(name="dbg", bufs=2)
                def ddump(dst, src, w):
                    for off in range(0, w, 1024):
                        ww = min(1024, w - off)
                        tl = dpool.tile([128, 1024], F32, tag="d", name="d")
                        nc.vector.tensor_copy(tl[:, :ww], src[:, off:off + ww])
                        nc.sync.dma_start(out=dst[:, off:off + ww],
                                          in_=tl[:, :ww])
                for hp in range(4):
                    ddump(dbg_ao[hp], aoT[hp], N)
                    ddump(dbg_sel[hp], sel[hp], TQ)
                for t in range(8):
                    ddump(dbg_ya[t], y_acc[t], DIM)
                ddump(dbg_kq[0], kt_cur, N)
                ddump(dbg_kq[1], qt_cur, N)
                ddump(dbg_v, v_t[0], 8 * 65)
                dpool.release()

    return nc


# ---------------------------------------------------------------------------
# Host side
# ---------------------------------------------------------------------------

_NC_CACHE = {}


def _get_nc(apply_gamma, apply_beta):
    key = (apply_gamma, apply_beta)
    if key not in _NC_CACHE:
        nc = build_nc(apply_gamma=apply_gamma, apply_beta=apply_beta)
        split_excess_waits(nc)
        _NC_CACHE[key] = nc
    return _NC_CACHE[key]


def make_in_maps(x, w_qkv, w_out, b_out, ln_gamma, ln_beta,
                 apply_gamma, apply_beta):
    bf = ml_dtypes.bfloat16
    B, N, DIM = x.shape
    INNER = w_qkv.shape[1] // 3
    TQ = N // 2
    ident = np.eye(128, dtype=bf)
    in_maps = []
    for c in range(8):
        b, g = c // 2, c % 2
        xb = x[b]
        if g == 1:
            xb = np.concatenate([xb[TQ:], xb[:TQ]], axis=0)
        col0 = 512 * g
        wo_re = np.concatenate([w_out[col0:col0 + 512],
                                w_out[512 - col0:1024 - col0]], axis=0)
        m = {
            "xT": np.ascontiguousarray(xb.T).astype(bf),
            "w_q": np.ascontiguousarray(w_qkv[:, col0:col0 + 512]).astype(bf),
            "w_k": np.ascontiguousarray(
                w_qkv[:, INNER + col0:INNER + col0 + 512]).astype(bf),
            "w_v": np.ascontiguousarray(
                w_qkv[:, 2 * INNER + col0:2 * INNER + col0 + 512]).astype(bf),
            "wo_re": np.ascontiguousarray(wo_re).astype(bf),
            "b_out": np.ascontiguousarray(b_out).astype(bf),
            "ident": ident,
            # AG output rows 0:128 = low rank's data, 128:256 = high rank's;
            # select the PEER: low core (g=0) takes rows 128:256 and vice versa
            "sel0": np.full((128, 1), float(g), dtype=np.float32),
            "sel1": np.full((128, 1), 1.0 - g, dtype=np.float32),
        }
        if apply_gamma:
            m["ln_gamma"] = np.ascontiguousarray(ln_gamma).astype(np.float32)
        if apply_beta:
            m["ln_beta"] = np.ascontiguousarray(ln_beta).astype(np.float32)
        in_maps.append(m)
    return in_maps


def kernel(x, w_qkv, w_out, b_out, ln_gamma, ln_beta):
    x = np.asarray(x, dtype=np.float32)
    w_qkv = np.asarray(w_qkv, dtype=np.float32)
    w_out = np.asarray(w_out, dtype=np.float32)
    b_out = np.asarray(b_out, dtype=np.float32)
    ln_gamma = np.asarray(ln_gamma, dtype=np.float32)
    ln_beta = np.asarray(ln_beta, dtype=np.float32)
    B, N, DIM = x.shape
    TQ = N // 2
    apply_gamma = not np.all(ln_gamma == 1.0)
    apply_beta = not np.all(ln_beta == 0.0)
    nc = _get_nc(apply_gamma, apply_beta)
    in_maps = make_in_maps(x, w_qkv, w_out, b_out, ln_gamma, ln_beta,
                           apply_gamma, apply_beta)
    res = run_bass_kernel_spmd(nc, in_maps, list(range(8)))
    out = np.empty((B, N, DIM), np.float32)
    for c in range(8):
        b, g = c // 2, c % 2
        out[b, g * TQ:(g + 1) * TQ] = res.results[c]["y"]
    return out

